# revision 1
# baseline (speedup 1.0000x reference)
"""Trainium2 Bass kernel for CrossAttention (LayerNorm + self-attention + 1x1 conv + residual).

Sharding: data-parallel over batch — B=8, one batch element per NeuronCore.
Per-core layout is feature-major ([C|HID partitions, L free]); the LayerNorm is
folded into the QKV projections via augmented contraction rows, softmax runs
without max-subtraction (logits are O(1)), and the denominator is accumulated
on the TensorEngine with col-tiled ones-matmuls.
"""
import numpy as np

B, C, L = 8, 256, 2048
H, DH = 4, 32
HID = H * DH           # 128
EPS = 1e-5
SCALE = DH ** -0.5
P = 128                # partitions
NL = L // 512          # 4 free-dim chunks of 512
NE = L // P            # 16 contraction chunks of 128

_cached = None


def _build():
    import concourse.bass as bass
    import concourse.bacc as bacc
    import concourse.tile as tile
    from concourse import mybir
    from concourse.masks import make_identity

    f32 = mybir.dt.float32
    AF = mybir.ActivationFunctionType
    OP = mybir.AluOpType

    nc = bacc.Bacc('TRN2', target_bir_lowering=False, debug=False, num_devices=B)

    xd = nc.dram_tensor('x', [C, L], f32, kind='ExternalInput').ap()
    gd = nc.dram_tensor('g', [C, 1], f32, kind='ExternalInput').ap()
    bd = nc.dram_tensor('b', [C, 1], f32, kind='ExternalInput').ap()
    wqd = nc.dram_tensor('Wq', [HID, C], f32, kind='ExternalInput').ap()
    wkd = nc.dram_tensor('Wk', [HID, C], f32, kind='ExternalInput').ap()
    wvd = nc.dram_tensor('Wv', [HID, C], f32, kind='ExternalInput').ap()
    wod = nc.dram_tensor('Wo', [C, HID], f32, kind='ExternalInput').ap()
    bod = nc.dram_tensor('bo', [C, 1], f32, kind='ExternalInput').ap()
    yd = nc.dram_tensor('y', [C, L], f32, kind='ExternalOutput').ap()

    with tile.TileContext(nc) as tc:
        with (
            tc.tile_pool(name='const', bufs=1) as const,
            tc.tile_pool(name='big', bufs=1) as big,
            tc.tile_pool(name='scratch', bufs=2) as scratch,
            tc.tile_pool(name='apool', bufs=2) as apool,
            tc.tile_pool(name='opool', bufs=2) as opool,
            tc.tile_pool(name='psBig', bufs=1, space='PSUM') as psBig,
            tc.tile_pool(name='psMid', bufs=2, space='PSUM') as psMid,
            tc.tile_pool(name='psSmall', bufs=2, space='PSUM') as psSmall,
        ):
            # ---- loads ----
            x0 = big.tile([P, L], f32, tag='x0')
            x1 = big.tile([P, L], f32, tag='x1')
            nc.sync.dma_start(out=x0, in_=xd[0:P, :])
            nc.sync.dma_start(out=x1, in_=xd[P:C, :])

            wq_nat = const.tile([HID, C], f32, tag='wq_nat')
            wk_nat = const.tile([HID, C], f32, tag='wk_nat')
            wv_nat = const.tile([HID, C], f32, tag='wv_nat')
            wo_nat = [const.tile([P, HID], f32, tag=f'wo_nat{c}', name=f'wo_nat{c}') for c in range(2)]
            nc.sync.dma_start(out=wq_nat, in_=wqd)
            nc.sync.dma_start(out=wk_nat, in_=wkd)
            nc.sync.dma_start(out=wv_nat, in_=wvd)
            for c in range(2):
                nc.sync.dma_start(out=wo_nat[c], in_=wod[c * P:(c + 1) * P, :])

            gc = [const.tile([P, 1], f32, tag=f'g{c}', name=f'g{c}') for c in range(2)]
            bc = [const.tile([P, 1], f32, tag=f'b{c}', name=f'b{c}') for c in range(2)]
            boc = [const.tile([P, 1], f32, tag=f'bo{c}', name=f'bo{c}') for c in range(2)]
            for c in range(2):
                nc.sync.dma_start(out=gc[c], in_=gd[c * P:(c + 1) * P, :])
                nc.sync.dma_start(out=bc[c], in_=bd[c * P:(c + 1) * P, :])
                nc.sync.dma_start(out=boc[c], in_=bod[c * P:(c + 1) * P, :])

            ident = const.tile([P, P], f32, tag='ident')
            make_identity(nc, ident)
            ones = const.tile([P, P], f32, tag='ones')
            nc.gpsimd.memset(ones, 1.0)
            epst = const.tile([P, 1], f32, tag='epst')
            nc.vector.memset(epst, EPS)
            neg_gc = [const.tile([P, 1], f32, tag=f'ng{c}', name=f'ng{c}') for c in range(2)]
            for c in range(2):
                nc.vector.tensor_scalar_mul(neg_gc[c], gc[c], -1.0)

            # ---- weight prep: transpose QKV weights to [C, HID], Wo to [HID, C] ----
            wT = {}
            for name, nat in (('q', wq_nat), ('k', wk_nat), ('v', wv_nat)):
                for c in range(2):
                    tp = psSmall.tile([P, P], f32, tag='sm')
                    nc.tensor.transpose(tp, nat[:, c * P:(c + 1) * P], ident)
                    t = const.tile([P, HID], f32, tag=f'w{name}T{c}', name=f'w{name}T{c}')
                    nc.vector.tensor_copy(t, tp)
                    wT[(name, c)] = t
            woT = const.tile([HID, C], f32, tag='woT')
            for c in range(2):
                tp = psSmall.tile([P, P], f32, tag='sm')
                nc.tensor.transpose(tp, wo_nat[c], ident)
                nc.vector.tensor_copy(woT[:, c * P:(c + 1) * P], tp)

            # ---- augmentation rows: row0 = -s?g, row1 = b?  (direct M=2 matmuls) ----
            # s?g[h] = sum_c W[h,c]*g[c], b?[h] = sum_c W[h,c]*b[c]
            augin = []
            for c in range(2):
                ai = const.tile([P, 2], f32, tag=f'augin{c}', name=f'augin{c}')
                nc.vector.tensor_copy(ai[:, 0:1], neg_gc[c])
                nc.vector.tensor_copy(ai[:, 1:2], bc[c])
                augin.append(ai)
            augT = {}
            for name in ('q', 'k', 'v'):
                ap_ = psSmall.tile([2, P], f32, tag='sm', name=f'augps{name}')
                for c in range(2):
                    nc.tensor.matmul(ap_, lhsT=augin[c], rhs=wT[(name, c)],
                                     start=(c == 0), stop=(c == 1))
                t = const.tile([2, P], f32, tag=f'augT{name}', name=f'augT{name}')
                nc.vector.tensor_copy(t, ap_)
                augT[name] = t
            # scale transposed QKV weights by g (per-partition in [C,HID] layout)
            for name in ('q', 'k', 'v'):
                for c in range(2):
                    nc.vector.tensor_scalar_mul(wT[(name, c)], wT[(name, c)], gc[c])

            # ---- LayerNorm statistics (replicated across partitions via ones-matmul) ----
            xsq0 = scratch.tile([P, L], f32, tag='sc')
            xsq1 = scratch.tile([P, L], f32, tag='sc')
            nc.vector.tensor_mul(xsq0, x0, x0)
            nc.vector.tensor_mul(xsq1, x1, x1)

            s1p = psBig.tile([P, L], f32, tag='ps')
            for n in range(NL):
                sl = slice(n * 512, (n + 1) * 512)
                nc.tensor.matmul(s1p[:, sl], lhsT=ones, rhs=x0[:, sl], start=True, stop=False)
                nc.tensor.matmul(s1p[:, sl], lhsT=ones, rhs=x1[:, sl], start=False, stop=True)
            mean_bc = big.tile([P, L], f32, tag='mean')
            nc.vector.tensor_scalar_mul(mean_bc, s1p, 1.0 / C)

            s2p = psBig.tile([P, L], f32, tag='ps')
            for n in range(NL):
                sl = slice(n * 512, (n + 1) * 512)
                nc.tensor.matmul(s2p[:, sl], lhsT=ones, rhs=xsq0[:, sl], start=True, stop=False)
                nc.tensor.matmul(s2p[:, sl], lhsT=ones, rhs=xsq1[:, sl], start=False, stop=True)
            msq = scratch.tile([P, L], f32, tag='sc')
            nc.vector.tensor_mul(msq, mean_bc, mean_bc)
            veps = scratch.tile([P, L], f32, tag='sc')
            nc.vector.scalar_tensor_tensor(veps, in0=s2p, scalar=1.0 / C, in1=msq,
                                           op0=OP.mult, op1=OP.subtract)
            # rstd = exp(-0.5*ln(var+eps)) — keeps everything in the ln/exp table set
            lnv = scratch.tile([P, L], f32, tag='sc')
            nc.scalar.activation(lnv, veps, AF.Ln, bias=epst)
            rstd_bc = big.tile([P, L], f32, tag='rstd')
            nc.scalar.activation(rstd_bc, lnv, AF.Exp, scale=-0.5)

            xs0 = big.tile([P, L], f32, tag='xs0')
            xs1 = big.tile([P, L], f32, tag='xs1')
            nc.vector.tensor_mul(xs0, x0, rstd_bc)
            nc.vector.tensor_mul(xs1, x1, rstd_bc)
            aug2 = const.tile([2, L], f32, tag='aug2')
            nc.gpsimd.memset(aug2, 1.0)
            nc.vector.tensor_mul(aug2[0:1, :], mean_bc[0:1, :], rstd_bc[0:1, :])

            # ---- QKV projections (feature-major QT/KT, position-major V) ----
            qt = big.tile([HID, L], f32, tag='qt')
            kt = big.tile([HID, L], f32, tag='kt')
            for name, dst in (('q', qt), ('k', kt)):
                pp = psBig.tile([P, L], f32, tag='ps')
                for n in range(NL):
                    sl = slice(n * 512, (n + 1) * 512)
                    nc.tensor.matmul(pp[:, sl], lhsT=wT[(name, 0)], rhs=xs0[:, sl], start=True, stop=False)
                    nc.tensor.matmul(pp[:, sl], lhsT=wT[(name, 1)], rhs=xs1[:, sl], start=False, stop=False)
                    nc.tensor.matmul(pp[:, sl], lhsT=augT[name], rhs=aug2[:, sl], start=False, stop=True)
                nc.vector.tensor_copy(dst, pp)
            vsb = big.tile([P, NE, HID], f32, tag='vsb')
            for e in range(NE):
                se = slice(e * P, (e + 1) * P)
                vp = psSmall.tile([P, HID], f32, tag='sm')
                nc.tensor.matmul(vp, lhsT=xs0[:, se], rhs=wT[('v', 0)], start=True, stop=False)
                nc.tensor.matmul(vp, lhsT=xs1[:, se], rhs=wT[('v', 1)], start=False, stop=False)
                nc.tensor.matmul(vp, lhsT=aug2[:, se], rhs=augT['v'], start=False, stop=True)
                nc.vector.tensor_copy(vsb[:, e, :], vp)

            # ---- attention: S^T -> exp -> (Z, attn@v) -> normalize -> out proj ----
            for d in range(NL):
                sd = slice(d * 512, (d + 1) * 512)
                zp = psMid.tile([P, 512], f32, tag='mid')
                op_ = psMid.tile([P, 512], f32, tag='mid')
                for e in range(NE):
                    se = slice(e * P, (e + 1) * P)
                    sp = psBig.tile([P, L], f32, tag='ps')
                    for h in range(H):
                        hp = slice(32 * h, 32 * h + 32)
                        sh = slice(512 * h, 512 * (h + 1))
                        nc.tensor.matmul(sp[:, sh], lhsT=kt[hp, se], rhs=qt[hp, sd],
                                         start=True, stop=True, tile_position=(32 * h, 0))
                    at = apool.tile([P, L], f32, tag='at')
                    nc.scalar.activation(at, sp, AF.Exp, scale=SCALE)
                    for h in range(H):
                        hp = slice(32 * h, 32 * h + 32)
                        sh = slice(512 * h, 512 * (h + 1))
                        nc.tensor.matmul(zp[hp, :], lhsT=ones[:, 0:32], rhs=at[:, sh],
                                         start=(e == 0), stop=(e == NE - 1),
                                         tile_position=(0, 32 * h))
                        nc.tensor.matmul(op_[hp, :], lhsT=vsb[:, e, hp], rhs=at[:, sh],
                                         start=(e == 0), stop=(e == NE - 1),
                                         tile_position=(0, 32 * h))
                rz = opool.tile([P, 512], f32, tag='rz')
                nc.vector.reciprocal(rz, zp)
                onorm = opool.tile([P, 512], f32, tag='onorm')
                nc.vector.tensor_mul(onorm, op_, rz)
                for c in range(2):
                    yp = psMid.tile([P, 512], f32, tag='mid')
                    nc.tensor.matmul(yp, lhsT=woT[:, c * P:(c + 1) * P], rhs=onorm,
                                     start=True, stop=True)
                    ysb = opool.tile([P, 512], f32, tag='ysb')
                    xc = x0 if c == 0 else x1
                    nc.vector.scalar_tensor_tensor(ysb, in0=yp, scalar=boc[c], in1=xc[:, sd],
                                                   op0=OP.add, op1=OP.add)
                    nc.sync.dma_start(out=yd[c * P:(c + 1) * P, sd], in_=ysb)

    nc.compile()
    return nc


def _get_nc():
    global _cached
    if _cached is None:
        _cached = _build()
    return _cached


def kernel(**inputs):
    from concourse.bass_utils import run_bass_kernel_spmd

    x = np.ascontiguousarray(np.asarray(inputs['x'], dtype=np.float32))
    g = np.asarray(inputs['g'], dtype=np.float32).reshape(C, 1)
    b = np.asarray(inputs['b'], dtype=np.float32).reshape(C, 1)
    wq = np.ascontiguousarray(np.asarray(inputs['Wq'], dtype=np.float32))
    wk = np.ascontiguousarray(np.asarray(inputs['Wk'], dtype=np.float32))
    wv = np.ascontiguousarray(np.asarray(inputs['Wv'], dtype=np.float32))
    wo = np.ascontiguousarray(np.asarray(inputs['Wo'], dtype=np.float32))
    bo = np.asarray(inputs['bo'], dtype=np.float32).reshape(C, 1)

    nc = _get_nc()
    in_maps = [
        {'x': x[i], 'g': g, 'b': b, 'Wq': wq, 'Wk': wk, 'Wv': wv, 'Wo': wo, 'bo': bo}
        for i in range(B)
    ]
    res = run_bass_kernel_spmd(nc, in_maps, list(range(B)))
    return np.stack([res.results[i]['y'] for i in range(B)]).astype(np.float32)



# revision 6
# speedup vs baseline: 1.7856x; 1.7856x over previous
"""Trainium2 Bass kernel for CrossAttention (LayerNorm + self-attention + 1x1 conv + residual).

Sharding: data-parallel over batch — B=8, one batch element per NeuronCore.
Feature-major layout ([C|HID partitions, L free]); LayerNorm folded into QKV via
augmented contraction rows; softmax without max-subtraction (logits are O(1)).

Perf notes vs v1:
- All heavy matmuls run in bf16 (1 cyc/row vs 4 for fp32); LN stats matmuls in fp32r.
- Softmax denominator Z is folded into the attnV matmul as extra `ones` columns of
  the stationary operand ([v_h | ones] -> rows 0-31 attnV_h, rows 32-63 Z_h), which
  eliminates the dedicated ones-matmul streams of v1.
- The partition misalignment (op rows 0-31 need 1/Z living at rows 32-63) is fixed
  with a small SBUF->SBUF DMA realign + band DVE multiplies (matmul outputs must
  start at PSUM partition 0, so each head gets its own [64, 512] accumulator).
- exp() is issued as [128, 1024] activations out of a 2-deep PSUM ring so the
  Scalar engine (the ~130us floor: 16.8M logits @ 1 elem/cycle/lane) streams
  back-to-back while PE runs ahead.
"""
import numpy as np

B, C, L = 8, 256, 2048
H, DH = 4, 32
HID = H * DH           # 128
EPS = 1e-5
SCALE = DH ** -0.5
P = 128                # partitions
NL = L // 512          # 4 query blocks of 512
NE = L // P            # 16 key blocks of 128

_cached = None


def _build():
    import concourse.bass as bass
    import concourse.bacc as bacc
    import concourse.tile as tile
    from concourse import mybir
    from concourse.masks import make_identity

    f32 = mybir.dt.float32
    f32r = mybir.dt.float32r
    bf16 = mybir.dt.bfloat16
    AF = mybir.ActivationFunctionType
    OP = mybir.AluOpType

    nc = bacc.Bacc('TRN2', target_bir_lowering=False, debug=False, num_devices=B)

    xd = nc.dram_tensor('x', [C, L], f32, kind='ExternalInput').ap()
    gd = nc.dram_tensor('g', [C, 1], f32, kind='ExternalInput').ap()
    bd = nc.dram_tensor('b', [C, 1], f32, kind='ExternalInput').ap()
    wqd = nc.dram_tensor('Wq', [HID, C], f32, kind='ExternalInput').ap()
    wkd = nc.dram_tensor('Wk', [HID, C], f32, kind='ExternalInput').ap()
    wvd = nc.dram_tensor('Wv', [HID, C], f32, kind='ExternalInput').ap()
    wod = nc.dram_tensor('Wo', [C, HID], f32, kind='ExternalInput').ap()
    bod = nc.dram_tensor('bo', [C, 1], f32, kind='ExternalInput').ap()
    yd = nc.dram_tensor('y', [C, L], f32, kind='ExternalOutput').ap()

    with tile.TileContext(nc) as tc:
        with (
            tc.tile_pool(name='const', bufs=1) as const,
            tc.tile_pool(name='big', bufs=1) as big,
            tc.tile_pool(name='scratch', bufs=2) as scratch,
            tc.tile_pool(name='scb', bufs=4) as scbp,
            tc.tile_pool(name='apool', bufs=3) as apool,
            tc.tile_pool(name='psSP', bufs=2, space='PSUM') as psSP,
            tc.tile_pool(name='psAcc', bufs=4, space='PSUM') as psAcc,
        ):
            # ---- loads ----
            x0 = big.tile([P, L], f32, tag='x0')
            x1 = big.tile([P, L], f32, tag='x1')
            nc.sync.dma_start(out=x0, in_=xd[0:P, :])
            nc.sync.dma_start(out=x1, in_=xd[P:C, :])

            wq_nat = const.tile([HID, C], f32, tag='wq_nat')
            wk_nat = const.tile([HID, C], f32, tag='wk_nat')
            wv_nat = const.tile([HID, C], f32, tag='wv_nat')
            wo_nat = [const.tile([P, HID], f32, tag=f'wo_nat{c}', name=f'wo_nat{c}') for c in range(2)]
            nc.sync.dma_start(out=wq_nat, in_=wqd)
            nc.sync.dma_start(out=wk_nat, in_=wkd)
            nc.sync.dma_start(out=wv_nat, in_=wvd)
            for c in range(2):
                nc.sync.dma_start(out=wo_nat[c], in_=wod[c * P:(c + 1) * P, :])

            gc = [const.tile([P, 1], f32, tag=f'g{c}', name=f'g{c}') for c in range(2)]
            bc = [const.tile([P, 1], f32, tag=f'b{c}', name=f'b{c}') for c in range(2)]
            boc = [const.tile([P, 1], f32, tag=f'bo{c}', name=f'bo{c}') for c in range(2)]
            for c in range(2):
                nc.sync.dma_start(out=gc[c], in_=gd[c * P:(c + 1) * P, :])
                nc.sync.dma_start(out=bc[c], in_=bd[c * P:(c + 1) * P, :])
                nc.sync.dma_start(out=boc[c], in_=bod[c * P:(c + 1) * P, :])

            ident = const.tile([P, P], f32, tag='ident')
            make_identity(nc, ident)
            ones_b = const.tile([P, P], bf16, tag='ones_b')
            nc.gpsimd.memset(ones_b, 1.0)
            epst = const.tile([P, 1], f32, tag='epst')
            nc.vector.memset(epst, EPS)
            neg_gc = [const.tile([P, 1], f32, tag=f'ng{c}', name=f'ng{c}') for c in range(2)]
            for c in range(2):
                nc.vector.tensor_scalar_mul(neg_gc[c], gc[c], -1.0)

            # ---- weight prep ----
            # QKV weights: transpose to [C, HID], scale by g, cast to bf16
            wT = {}
            for name, nat in (('q', wq_nat), ('k', wk_nat), ('v', wv_nat)):
                for c in range(2):
                    tp = psAcc.tile([P, P], f32, tag='fold')
                    nc.tensor.transpose(tp, nat[:, c * P:(c + 1) * P], ident)
                    t = const.tile([P, HID], bf16, tag=f'w{name}T{c}', name=f'w{name}T{c}')
                    nc.vector.tensor_scalar_mul(t, tp, gc[c])
                    wT[(name, c)] = t

            # Wo^T per head: woT_h[h] = [32 hid-ch, 256 out-ch] bf16
            woT_h = [const.tile([32, C], bf16, tag=f'woT_h{h}', name=f'woT_h{h}') for h in range(H)]
            for c in range(2):
                for h in range(H):
                    tp = psAcc.tile([32, P], f32, tag='fold')
                    nc.tensor.transpose(tp, wo_nat[c][:, h * 32:(h + 1) * 32], ident)
                    nc.vector.tensor_copy(woT_h[h][:, c * P:(c + 1) * P], tp)

            # ---- augmentation rows: row0 = -s?g, row1 = b?  ----
            augin = []
            for c in range(2):
                ai = const.tile([P, 2], bf16, tag=f'augin{c}', name=f'augin{c}')
                nc.vector.tensor_copy(ai[:, 0:1], neg_gc[c])
                nc.vector.tensor_copy(ai[:, 1:2], bc[c])
                augin.append(ai)
            augT = {}
            for name in ('q', 'k', 'v'):
                ap_ = psAcc.tile([2, P], f32, tag='fold', name=f'augps{name}')
                for c in range(2):
                    nc.tensor.matmul(ap_, lhsT=augin[c], rhs=wT[(name, c)],
                                     start=(c == 0), stop=(c == 1))
                t = const.tile([2, P], bf16, tag=f'augT{name}', name=f'augT{name}')
                nc.vector.tensor_copy(t, ap_)
                augT[name] = t

            # ---- LayerNorm statistics (replicated across partitions via ones-matmul) ----
            xb0 = scbp.tile([P, L], bf16, tag='scb')
            xb1 = scbp.tile([P, L], bf16, tag='scb')
            nc.vector.tensor_copy(xb0, x0)
            nc.vector.tensor_copy(xb1, x1)
            xsq0 = scbp.tile([P, L], bf16, tag='scb')
            xsq1 = scbp.tile([P, L], bf16, tag='scb')
            nc.vector.tensor_mul(xsq0, xb0, xb0)
            nc.vector.tensor_mul(xsq1, xb1, xb1)

            mean_bc = big.tile([P, L], f32, tag='mean')
            veps = scratch.tile([P, L], f32, tag='sc')
            msq = scratch.tile([P, L], f32, tag='sc')
            for n in range(2):
                hs = slice(n * 1024, (n + 1) * 1024)
                s1p = psSP.tile([P, 1024], f32, tag='sp')
                for m in range(2):
                    sl = slice(m * 512, (m + 1) * 512)
                    gsl = slice(n * 1024 + m * 512, n * 1024 + (m + 1) * 512)
                    nc.tensor.matmul(s1p[:, sl], lhsT=ones_b, rhs=xb0[:, gsl], start=True, stop=False)
                    nc.tensor.matmul(s1p[:, sl], lhsT=ones_b, rhs=xb1[:, gsl], start=False, stop=True)
                nc.vector.tensor_scalar_mul(mean_bc[:, hs], s1p, 1.0 / C)
                nc.vector.tensor_mul(msq[:, hs], mean_bc[:, hs], mean_bc[:, hs])
                s2p = psSP.tile([P, 1024], f32, tag='sp')
                for m in range(2):
                    sl = slice(m * 512, (m + 1) * 512)
                    gsl = slice(n * 1024 + m * 512, n * 1024 + (m + 1) * 512)
                    nc.tensor.matmul(s2p[:, sl], lhsT=ones_b, rhs=xsq0[:, gsl], start=True, stop=False)
                    nc.tensor.matmul(s2p[:, sl], lhsT=ones_b, rhs=xsq1[:, gsl], start=False, stop=True)
                nc.vector.scalar_tensor_tensor(veps[:, hs], in0=s2p, scalar=1.0 / C,
                                               in1=msq[:, hs], op0=OP.mult, op1=OP.subtract)
            # rstd = exp(-0.5*ln(var+eps)) — stays in the ln/exp table set
            lnv = scratch.tile([P, L], f32, tag='sc')
            nc.scalar.activation(lnv, veps, AF.Ln, bias=epst)
            rstd_bc = big.tile([P, L], f32, tag='rstd')
            nc.scalar.activation(rstd_bc, lnv, AF.Exp, scale=-0.5)

            xs0 = big.tile([P, L], bf16, tag='xs0')
            xs1 = big.tile([P, L], bf16, tag='xs1')
            nc.vector.tensor_mul(xs0, x0, rstd_bc)
            nc.vector.tensor_mul(xs1, x1, rstd_bc)
            aug2 = const.tile([2, L], bf16, tag='aug2')
            nc.gpsimd.memset(aug2, 1.0)
            nc.vector.tensor_mul(aug2[0:1, :], mean_bc[0:1, :], rstd_bc[0:1, :])

            # ---- QKV projections (feature-major QT/KT; V position-major with ones fold) ----
            qt = big.tile([HID, L], bf16, tag='qt')
            kt = big.tile([HID, L], bf16, tag='kt')
            for name, dst in (('q', qt), ('k', kt)):
                for n in range(2):
                    pp = psSP.tile([P, 1024], f32, tag='sp')
                    for m in range(2):
                        sl = slice(m * 512, (m + 1) * 512)
                        gsl = slice(n * 1024 + m * 512, n * 1024 + (m + 1) * 512)
                        nc.tensor.matmul(pp[:, sl], lhsT=wT[(name, 0)], rhs=xs0[:, gsl], start=True, stop=False)
                        nc.tensor.matmul(pp[:, sl], lhsT=wT[(name, 1)], rhs=xs1[:, gsl], start=False, stop=False)
                        nc.tensor.matmul(pp[:, sl], lhsT=augT[name], rhs=aug2[:, gsl], start=False, stop=True)
                    nc.vector.tensor_copy(dst[:, n * 1024:(n + 1) * 1024], pp)

            # vext[:, e, h, 0:32] = V channels of head h at positions of block e;
            # vext[:, e, h, 32:64] = 1.0 (the Z fold columns)
            vext = big.tile([P, NE, H, 64], bf16, tag='vext')
            nc.gpsimd.memset(vext, 1.0)
            for e in range(NE):
                se = slice(e * P, (e + 1) * P)
                vp = psSP.tile([P, HID], f32, tag='sp')
                nc.tensor.matmul(vp, lhsT=xs0[:, se], rhs=wT[('v', 0)], start=True, stop=False)
                nc.tensor.matmul(vp, lhsT=xs1[:, se], rhs=wT[('v', 1)], start=False, stop=False)
                nc.tensor.matmul(vp, lhsT=aug2[:, se], rhs=augT['v'], start=False, stop=True)
                nc.vector.tensor_copy(vext[:, e, :, 0:32], vp)

            # static tiles for the per-d tail
            rcp = [const.tile([64, 512], f32, tag=f'rcp{h}', name=f'rcp{h}') for h in range(H)]
            rz = [const.tile([32, 512], f32, tag=f'rz{h}', name=f'rz{h}') for h in range(H)]
            onorm = [const.tile([32, 512], bf16, tag=f'onorm{h}', name=f'onorm{h}') for h in range(H)]

            # ---- attention ----
            for d in range(NL):
                sd = slice(d * 512, (d + 1) * 512)
                # per-head fold accumulators: rows 0-31 attnV_h, rows 32-63 Z_h (x32)
                acc = [psAcc.tile([64, 512], f32, tag='fold', name=f'acc{d}_{h}') for h in range(H)]
                for e in range(NE):
                    se = slice(e * P, (e + 1) * P)
                    for pair in range(2):
                        h0, h1 = 2 * pair, 2 * pair + 1
                        sp = psSP.tile([P, 1024], f32, tag='sp')
                        nc.tensor.matmul(sp[:, 0:512], lhsT=kt[h0 * 32:(h0 + 1) * 32, se],
                                         rhs=qt[h0 * 32:(h0 + 1) * 32, sd],
                                         start=True, stop=True, tile_position=(32 * h0, 0))
                        nc.tensor.matmul(sp[:, 512:1024], lhsT=kt[h1 * 32:(h1 + 1) * 32, se],
                                         rhs=qt[h1 * 32:(h1 + 1) * 32, sd],
                                         start=True, stop=True, tile_position=(32 * h1, 0))
                        at = apool.tile([P, 1024], bf16, tag='at')
                        nc.scalar.activation(at, sp, AF.Exp, scale=SCALE)
                        nc.tensor.matmul(acc[h0], lhsT=vext[:, e, h0, :], rhs=at[:, 0:512],
                                         start=(e == 0), stop=(e == NE - 1),
                                         tile_position=(0, 0))
                        nc.tensor.matmul(acc[h1], lhsT=vext[:, e, h1, :], rhs=at[:, 512:1024],
                                         start=(e == 0), stop=(e == NE - 1),
                                         tile_position=(0, 0))
                # tail: 1/Z, realign Z band down 32 partitions, normalize, project
                for h in range(H):
                    nc.vector.reciprocal(rcp[h][32:64, :], acc[h][32:64, :])
                    nc.sync.dma_start(out=rz[h], in_=rcp[h][32:64, :])
                    nc.vector.tensor_mul(onorm[h], acc[h][0:32, :], rz[h])
                for c in range(2):
                    yp = psSP.tile([P, 512], f32, tag='sp')
                    for h in range(H):
                        nc.tensor.matmul(yp, lhsT=woT_h[h][:, c * P:(c + 1) * P],
                                         rhs=onorm[h], start=(h == 0), stop=(h == H - 1))
                    ysb = scratch.tile([P, 512], f32, tag='ysb')
                    xc = x0 if c == 0 else x1
                    nc.vector.scalar_tensor_tensor(ysb, in0=yp, scalar=boc[c],
                                                   in1=xc[:, sd], op0=OP.add, op1=OP.add)
                    nc.sync.dma_start(out=yd[c * P:(c + 1) * P, sd], in_=ysb)

    nc.compile()
    return nc


def _get_nc():
    global _cached
    if _cached is None:
        _cached = _build()
    return _cached


def kernel(**inputs):
    from concourse.bass_utils import run_bass_kernel_spmd

    x = np.ascontiguousarray(np.asarray(inputs['x'], dtype=np.float32))
    g = np.asarray(inputs['g'], dtype=np.float32).reshape(C, 1)
    b = np.asarray(inputs['b'], dtype=np.float32).reshape(C, 1)
    wq = np.ascontiguousarray(np.asarray(inputs['Wq'], dtype=np.float32))
    wk = np.ascontiguousarray(np.asarray(inputs['Wk'], dtype=np.float32))
    wv = np.ascontiguousarray(np.asarray(inputs['Wv'], dtype=np.float32))
    wo = np.ascontiguousarray(np.asarray(inputs['Wo'], dtype=np.float32))
    bo = np.asarray(inputs['bo'], dtype=np.float32).reshape(C, 1)

    nc = _get_nc()
    in_maps = [
        {'x': x[i], 'g': g, 'b': b, 'Wq': wq, 'Wk': wk, 'Wv': wv, 'Wo': wo, 'bo': bo}
        for i in range(B)
    ]
    res = run_bass_kernel_spmd(nc, in_maps, list(range(B)))
    return np.stack([res.results[i]['y'] for i in range(B)]).astype(np.float32)


# revision 7
# speedup vs baseline: 2.4980x; 1.3990x over previous
"""Trainium2 Bass kernel for CrossAttention (LayerNorm + self-attention + 1x1 conv + residual).

Sharding: data-parallel over batch — B=8, one batch element per NeuronCore.
Feature-major layout ([C|HID partitions, L free]); LayerNorm folded into QKV via
augmented contraction rows; softmax without max-subtraction (logits are O(1)).

Perf notes vs v1:
- All heavy matmuls run in bf16 (1 cyc/row vs 4 for fp32); LN stats matmuls in fp32r.
- Softmax denominator Z is folded into the attnV matmul as extra `ones` columns of
  the stationary operand ([v_h | ones] -> rows 0-31 attnV_h, rows 32-63 Z_h), which
  eliminates the dedicated ones-matmul streams of v1.
- The partition misalignment (op rows 0-31 need 1/Z living at rows 32-63) is fixed
  with a small SBUF->SBUF DMA realign + band DVE multiplies (matmul outputs must
  start at PSUM partition 0, so each head gets its own [64, 512] accumulator).
- exp() is issued as [128, 1024] activations out of a 2-deep PSUM ring so the
  Scalar engine (the ~130us floor: 16.8M logits @ 1 elem/cycle/lane) streams
  back-to-back while PE runs ahead.
"""
import numpy as np

B, C, L = 8, 256, 2048
H, DH = 4, 32
HID = H * DH           # 128
EPS = 1e-5
SCALE = DH ** -0.5
P = 128                # partitions
NL = L // 512          # 4 query blocks of 512
NE = L // P            # 16 key blocks of 128

_cached = None


def _build():
    import concourse.bass as bass
    import concourse.bacc as bacc
    import concourse.tile as tile
    from concourse import mybir
    from concourse.masks import make_identity

    f32 = mybir.dt.float32
    f32r = mybir.dt.float32r
    bf16 = mybir.dt.bfloat16
    AF = mybir.ActivationFunctionType
    OP = mybir.AluOpType

    nc = bacc.Bacc('TRN2', target_bir_lowering=False, debug=False, num_devices=B)

    xd = nc.dram_tensor('x', [C, L], f32, kind='ExternalInput').ap()
    gd = nc.dram_tensor('g', [C, 1], f32, kind='ExternalInput').ap()
    bd = nc.dram_tensor('b', [C, 1], f32, kind='ExternalInput').ap()
    wqd = nc.dram_tensor('Wq', [HID, C], f32, kind='ExternalInput').ap()
    wkd = nc.dram_tensor('Wk', [HID, C], f32, kind='ExternalInput').ap()
    wvd = nc.dram_tensor('Wv', [HID, C], f32, kind='ExternalInput').ap()
    wod = nc.dram_tensor('Wo', [C, HID], f32, kind='ExternalInput').ap()
    bod = nc.dram_tensor('bo', [C, 1], f32, kind='ExternalInput').ap()
    yd = nc.dram_tensor('y', [C, L], f32, kind='ExternalOutput').ap()

    with tile.TileContext(nc) as tc:
        with (
            tc.tile_pool(name='const', bufs=1) as const,
            tc.tile_pool(name='big', bufs=1) as big,
            tc.tile_pool(name='scratch', bufs=2) as scratch,
            tc.tile_pool(name='scb', bufs=4) as scbp,
            tc.tile_pool(name='apool', bufs=4) as apool,
            tc.tile_pool(name='psSP', bufs=3, space='PSUM') as psSP,
            tc.tile_pool(name='psAcc', bufs=2, space='PSUM') as psAcc,
        ):
            # ---- loads ----
            x0 = big.tile([P, L], f32, tag='x0')
            x1 = big.tile([P, L], f32, tag='x1')
            nc.sync.dma_start(out=x0, in_=xd[0:P, :])
            nc.sync.dma_start(out=x1, in_=xd[P:C, :])

            wq_nat = const.tile([HID, C], f32, tag='wq_nat')
            wk_nat = const.tile([HID, C], f32, tag='wk_nat')
            wv_nat = const.tile([HID, C], f32, tag='wv_nat')
            wo_nat = [const.tile([P, HID], f32, tag=f'wo_nat{c}', name=f'wo_nat{c}') for c in range(2)]
            nc.sync.dma_start(out=wq_nat, in_=wqd)
            nc.sync.dma_start(out=wk_nat, in_=wkd)
            nc.sync.dma_start(out=wv_nat, in_=wvd)
            for c in range(2):
                nc.sync.dma_start(out=wo_nat[c], in_=wod[c * P:(c + 1) * P, :])

            gc = [const.tile([P, 1], f32, tag=f'g{c}', name=f'g{c}') for c in range(2)]
            bc = [const.tile([P, 1], f32, tag=f'b{c}', name=f'b{c}') for c in range(2)]
            boc = [const.tile([P, 1], f32, tag=f'bo{c}', name=f'bo{c}') for c in range(2)]
            for c in range(2):
                nc.sync.dma_start(out=gc[c], in_=gd[c * P:(c + 1) * P, :])
                nc.sync.dma_start(out=bc[c], in_=bd[c * P:(c + 1) * P, :])
                nc.sync.dma_start(out=boc[c], in_=bod[c * P:(c + 1) * P, :])

            ident = const.tile([P, P], f32, tag='ident')
            make_identity(nc, ident)
            ones_b = const.tile([P, P], bf16, tag='ones_b')
            nc.gpsimd.memset(ones_b, 1.0)
            epst = const.tile([P, 1], f32, tag='epst')
            nc.vector.memset(epst, EPS)
            neg_gc = [const.tile([P, 1], f32, tag=f'ng{c}', name=f'ng{c}') for c in range(2)]
            for c in range(2):
                nc.vector.tensor_scalar_mul(neg_gc[c], gc[c], -1.0)

            # ---- weight prep ----
            # QKV weights: transpose to [C, HID], scale by g, cast to bf16
            wT = {}
            for name, nat in (('q', wq_nat), ('k', wk_nat), ('v', wv_nat)):
                for c in range(2):
                    tp = psAcc.tile([P, P], f32, tag='fold')
                    nc.tensor.transpose(tp, nat[:, c * P:(c + 1) * P], ident)
                    t = const.tile([P, HID], bf16, tag=f'w{name}T{c}', name=f'w{name}T{c}')
                    nc.vector.tensor_scalar_mul(t, tp, gc[c])
                    wT[(name, c)] = t

            # Wo^T in pair layout: woT_pair[i] [128, 256] bf16 with rows 0-31 =
            # head 2i channels, rows 64-95 = head 2i+1 channels, rest zero
            # (zeros null out the Z/garbage bands of onorm in the out-proj matmul).
            woT_pair = [const.tile([P, C], bf16, tag=f'woT_pair{i}', name=f'woT_pair{i}') for i in range(2)]
            wtmp = [const.tile([32, C], bf16, tag=f'wtmp{i}', name=f'wtmp{i}') for i in range(2)]
            for i in range(2):
                nc.vector.memset(woT_pair[i], 0.0)
            for c in range(2):
                for i in range(2):
                    tp = psAcc.tile([32, P], f32, tag='fold')
                    nc.tensor.transpose(tp, wo_nat[c][:, (2 * i) * 32:(2 * i + 1) * 32], ident)
                    nc.vector.tensor_copy(woT_pair[i][0:32, c * P:(c + 1) * P], tp)
                    tp2 = psAcc.tile([32, P], f32, tag='fold')
                    nc.tensor.transpose(tp2, wo_nat[c][:, (2 * i + 1) * 32:(2 * i + 2) * 32], ident)
                    nc.vector.tensor_copy(wtmp[i][:, c * P:(c + 1) * P], tp2)
            for i in range(2):
                nc.sync.dma_start(out=woT_pair[i][64:96, :], in_=wtmp[i])

            # ---- augmentation rows: row0 = -s?g, row1 = b?  ----
            augin = []
            for c in range(2):
                ai = const.tile([P, 2], bf16, tag=f'augin{c}', name=f'augin{c}')
                nc.vector.tensor_copy(ai[:, 0:1], neg_gc[c])
                nc.vector.tensor_copy(ai[:, 1:2], bc[c])
                augin.append(ai)
            augT = {}
            for name in ('q', 'k', 'v'):
                ap_ = psAcc.tile([2, P], f32, tag='fold', name=f'augps{name}')
                for c in range(2):
                    nc.tensor.matmul(ap_, lhsT=augin[c], rhs=wT[(name, c)],
                                     start=(c == 0), stop=(c == 1))
                t = const.tile([2, P], bf16, tag=f'augT{name}', name=f'augT{name}')
                nc.vector.tensor_copy(t, ap_)
                augT[name] = t

            # ---- LayerNorm statistics (replicated across partitions via ones-matmul) ----
            xb0 = scbp.tile([P, L], bf16, tag='scb')
            xb1 = scbp.tile([P, L], bf16, tag='scb')
            nc.vector.tensor_copy(xb0, x0)
            nc.vector.tensor_copy(xb1, x1)
            xsq0 = scbp.tile([P, L], bf16, tag='scb')
            xsq1 = scbp.tile([P, L], bf16, tag='scb')
            nc.vector.tensor_mul(xsq0, xb0, xb0)
            nc.vector.tensor_mul(xsq1, xb1, xb1)

            mean_bc = big.tile([P, L], f32, tag='mean')
            veps = scratch.tile([P, L], f32, tag='sc')
            msq = scratch.tile([P, L], f32, tag='sc')
            for n in range(2):
                hs = slice(n * 1024, (n + 1) * 1024)
                s1p = psSP.tile([P, 1024], f32, tag='sp')
                for m in range(2):
                    sl = slice(m * 512, (m + 1) * 512)
                    gsl = slice(n * 1024 + m * 512, n * 1024 + (m + 1) * 512)
                    nc.tensor.matmul(s1p[:, sl], lhsT=ones_b, rhs=xb0[:, gsl], start=True, stop=False)
                    nc.tensor.matmul(s1p[:, sl], lhsT=ones_b, rhs=xb1[:, gsl], start=False, stop=True)
                nc.vector.tensor_scalar_mul(mean_bc[:, hs], s1p, 1.0 / C)
                nc.vector.tensor_mul(msq[:, hs], mean_bc[:, hs], mean_bc[:, hs])
                s2p = psSP.tile([P, 1024], f32, tag='sp')
                for m in range(2):
                    sl = slice(m * 512, (m + 1) * 512)
                    gsl = slice(n * 1024 + m * 512, n * 1024 + (m + 1) * 512)
                    nc.tensor.matmul(s2p[:, sl], lhsT=ones_b, rhs=xsq0[:, gsl], start=True, stop=False)
                    nc.tensor.matmul(s2p[:, sl], lhsT=ones_b, rhs=xsq1[:, gsl], start=False, stop=True)
                nc.vector.scalar_tensor_tensor(veps[:, hs], in0=s2p, scalar=1.0 / C,
                                               in1=msq[:, hs], op0=OP.mult, op1=OP.subtract)
            # rstd = exp(-0.5*ln(var+eps)) — stays in the ln/exp table set
            lnv = scratch.tile([P, L], f32, tag='sc')
            nc.scalar.activation(lnv, veps, AF.Ln, bias=epst)
            rstd_bc = big.tile([P, L], f32, tag='rstd')
            nc.scalar.activation(rstd_bc, lnv, AF.Exp, scale=-0.5)

            xs0 = big.tile([P, L], bf16, tag='xs0')
            xs1 = big.tile([P, L], bf16, tag='xs1')
            nc.vector.tensor_mul(xs0, x0, rstd_bc)
            nc.vector.tensor_mul(xs1, x1, rstd_bc)
            aug2 = const.tile([2, L], bf16, tag='aug2')
            nc.gpsimd.memset(aug2, 1.0)
            nc.vector.tensor_mul(aug2[0:1, :], mean_bc[0:1, :], rstd_bc[0:1, :])

            # ---- QKV projections (feature-major QT/KT; V position-major with ones fold) ----
            qt = big.tile([HID, L], bf16, tag='qt')
            kt = big.tile([HID, L], bf16, tag='kt')
            for name, dst in (('q', qt), ('k', kt)):
                for n in range(2):
                    pp = psSP.tile([P, 1024], f32, tag='sp')
                    for m in range(2):
                        sl = slice(m * 512, (m + 1) * 512)
                        gsl = slice(n * 1024 + m * 512, n * 1024 + (m + 1) * 512)
                        nc.tensor.matmul(pp[:, sl], lhsT=wT[(name, 0)], rhs=xs0[:, gsl], start=True, stop=False)
                        nc.tensor.matmul(pp[:, sl], lhsT=wT[(name, 1)], rhs=xs1[:, gsl], start=False, stop=False)
                        nc.tensor.matmul(pp[:, sl], lhsT=augT[name], rhs=aug2[:, gsl], start=False, stop=True)
                    nc.vector.tensor_copy(dst[:, n * 1024:(n + 1) * 1024], pp)

            # vext[:, e, h, 0:32] = V channels of head h at positions of block e;
            # vext[:, e, h, 32:64] = 1.0 (the Z fold columns)
            vext = big.tile([P, NE, H, 64], bf16, tag='vext')
            nc.gpsimd.memset(vext, 1.0)
            for e in range(NE):
                se = slice(e * P, (e + 1) * P)
                vp = psSP.tile([P, HID], f32, tag='sp')
                nc.tensor.matmul(vp, lhsT=xs0[:, se], rhs=wT[('v', 0)], start=True, stop=False)
                nc.tensor.matmul(vp, lhsT=xs1[:, se], rhs=wT[('v', 1)], start=False, stop=False)
                nc.tensor.matmul(vp, lhsT=aug2[:, se], rhs=augT['v'], start=False, stop=True)
                nc.vector.tensor_copy(vext[:, e, :, 0:32], vp)

            # static tiles for the per-d tail (pair layout: rows 0-31 op_even,
            # 32-63 Z_even, 64-95 op_odd, 96-127 Z_odd)
            accsb = [const.tile([P, 512], f32, tag=f'accsb{i}', name=f'accsb{i}') for i in range(2)]
            rcp = [const.tile([P, 512], f32, tag=f'rcp{i}', name=f'rcp{i}') for i in range(2)]
            rz = [const.tile([P, 512], f32, tag=f'rz{i}', name=f'rz{i}') for i in range(2)]
            onorm = [const.tile([P, 512], bf16, tag=f'onorm{i}', name=f'onorm{i}') for i in range(2)]
            for i in range(2):
                nc.vector.memset(rz[i], 1.0)

            # ---- attention ----
            for d in range(NL):
                sd = slice(d * 512, (d + 1) * 512)
                # pair fold accumulators [128, 512]: head 2i at rows 0-63
                # (op 0-31, Z 32-63), head 2i+1 at rows 64-127 via tile_position
                acc = [psAcc.tile([P, 512], f32, tag='fold', name=f'acc{d}_{i}') for i in range(2)]
                for e in range(NE):
                    se = slice(e * P, (e + 1) * P)
                    for pair in range(2):
                        h0, h1 = 2 * pair, 2 * pair + 1
                        sp = psSP.tile([P, 1024], f32, tag='sp')
                        nc.tensor.matmul(sp[:, 0:512], lhsT=kt[h0 * 32:(h0 + 1) * 32, se],
                                         rhs=qt[h0 * 32:(h0 + 1) * 32, sd],
                                         start=True, stop=True, tile_position=(32 * h0, 0))
                        nc.tensor.matmul(sp[:, 512:1024], lhsT=kt[h1 * 32:(h1 + 1) * 32, se],
                                         rhs=qt[h1 * 32:(h1 + 1) * 32, sd],
                                         start=True, stop=True, tile_position=(32 * h1, 0))
                        at = apool.tile([P, 1024], bf16, tag='at')
                        nc.scalar.activation(at, sp, AF.Exp, scale=SCALE)
                        nc.tensor.matmul(acc[pair][0:64, :], lhsT=vext[:, e, h0, :],
                                         rhs=at[:, 0:512],
                                         start=(e == 0), stop=(e == NE - 1),
                                         tile_position=(0, 0))
                        nc.tensor.matmul(acc[pair][64:128, :], lhsT=vext[:, e, h1, :],
                                         rhs=at[:, 512:1024],
                                         start=(e == 0), stop=(e == NE - 1),
                                         tile_position=(0, 64))
                # tail: copy psum out early (frees acc for d+1), 1/Z via fast
                # approx, DMA-realign Z bands down 32 partitions, normalize
                for i in range(2):
                    nc.vector.tensor_copy(accsb[i], acc[i])
                    nc.vector.reciprocal_approx_fast(out=rcp[i], in_=accsb[i])
                    nc.sync.dma_start(out=rz[i][0:32, :], in_=rcp[i][32:64, :])
                    nc.sync.dma_start(out=rz[i][64:96, :], in_=rcp[i][96:128, :])
                    nc.vector.tensor_mul(onorm[i], accsb[i], rz[i])
                for c in range(2):
                    yp = psSP.tile([P, 512], f32, tag='sp')
                    nc.tensor.matmul(yp, lhsT=woT_pair[0][:, c * P:(c + 1) * P],
                                     rhs=onorm[0], start=True, stop=False)
                    nc.tensor.matmul(yp, lhsT=woT_pair[1][:, c * P:(c + 1) * P],
                                     rhs=onorm[1], start=False, stop=True)
                    ysb = scratch.tile([P, 512], f32, tag='ysb')
                    xc = x0 if c == 0 else x1
                    nc.vector.scalar_tensor_tensor(ysb, in0=yp, scalar=boc[c],
                                                   in1=xc[:, sd], op0=OP.add, op1=OP.add)
                    nc.sync.dma_start(out=yd[c * P:(c + 1) * P, sd], in_=ysb)

    nc.compile()
    return nc


def _get_nc():
    global _cached
    if _cached is None:
        _cached = _build()
    return _cached


def kernel(**inputs):
    from concourse.bass_utils import run_bass_kernel_spmd

    x = np.ascontiguousarray(np.asarray(inputs['x'], dtype=np.float32))
    g = np.asarray(inputs['g'], dtype=np.float32).reshape(C, 1)
    b = np.asarray(inputs['b'], dtype=np.float32).reshape(C, 1)
    wq = np.ascontiguousarray(np.asarray(inputs['Wq'], dtype=np.float32))
    wk = np.ascontiguousarray(np.asarray(inputs['Wk'], dtype=np.float32))
    wv = np.ascontiguousarray(np.asarray(inputs['Wv'], dtype=np.float32))
    wo = np.ascontiguousarray(np.asarray(inputs['Wo'], dtype=np.float32))
    bo = np.asarray(inputs['bo'], dtype=np.float32).reshape(C, 1)

    nc = _get_nc()
    in_maps = [
        {'x': x[i], 'g': g, 'b': b, 'Wq': wq, 'Wk': wk, 'Wv': wv, 'Wo': wo, 'bo': bo}
        for i in range(B)
    ]
    res = run_bass_kernel_spmd(nc, in_maps, list(range(B)))
    return np.stack([res.results[i]['y'] for i in range(B)]).astype(np.float32)


# revision 8
# speedup vs baseline: 2.5050x; 1.0028x over previous
"""Trainium2 Bass kernel for CrossAttention (LayerNorm + self-attention + 1x1 conv + residual).

Sharding: data-parallel over batch — B=8, one batch element per NeuronCore.
Feature-major layout ([C|HID partitions, L free]); LayerNorm folded into QKV via
augmented contraction rows; softmax without max-subtraction (logits are O(1)).

Perf notes vs v1:
- All heavy matmuls run in bf16 (1 cyc/row vs 4 for fp32); LN stats matmuls in fp32r.
- Softmax denominator Z is folded into the attnV matmul as extra `ones` columns of
  the stationary operand ([v_h | ones] -> rows 0-31 attnV_h, rows 32-63 Z_h), which
  eliminates the dedicated ones-matmul streams of v1.
- The partition misalignment (op rows 0-31 need 1/Z living at rows 32-63) is fixed
  with a small SBUF->SBUF DMA realign + band DVE multiplies (matmul outputs must
  start at PSUM partition 0, so each head gets its own [64, 512] accumulator).
- exp() is issued as [128, 1024] activations out of a 2-deep PSUM ring so the
  Scalar engine (the ~130us floor: 16.8M logits @ 1 elem/cycle/lane) streams
  back-to-back while PE runs ahead.
"""
import numpy as np

B, C, L = 8, 256, 2048
H, DH = 4, 32
HID = H * DH           # 128
EPS = 1e-5
SCALE = DH ** -0.5
P = 128                # partitions
NL = L // 512          # 4 query blocks of 512
NE = L // P            # 16 key blocks of 128

_cached = None


def _build():
    import concourse.bass as bass
    import concourse.bacc as bacc
    import concourse.tile as tile
    from concourse import mybir
    from concourse.masks import make_identity

    f32 = mybir.dt.float32
    f32r = mybir.dt.float32r
    bf16 = mybir.dt.bfloat16
    AF = mybir.ActivationFunctionType
    OP = mybir.AluOpType

    nc = bacc.Bacc('TRN2', target_bir_lowering=False, debug=False, num_devices=B)

    xd = nc.dram_tensor('x', [C, L], f32, kind='ExternalInput').ap()
    gd = nc.dram_tensor('g', [C, 1], f32, kind='ExternalInput').ap()
    bd = nc.dram_tensor('b', [C, 1], f32, kind='ExternalInput').ap()
    wqd = nc.dram_tensor('Wq', [HID, C], f32, kind='ExternalInput').ap()
    wkd = nc.dram_tensor('Wk', [HID, C], f32, kind='ExternalInput').ap()
    wvd = nc.dram_tensor('Wv', [HID, C], f32, kind='ExternalInput').ap()
    wod = nc.dram_tensor('Wo', [C, HID], f32, kind='ExternalInput').ap()
    bod = nc.dram_tensor('bo', [C, 1], f32, kind='ExternalInput').ap()
    yd = nc.dram_tensor('y', [C, L], f32, kind='ExternalOutput').ap()

    with tile.TileContext(nc) as tc:
        with (
            tc.tile_pool(name='const', bufs=1) as const,
            tc.tile_pool(name='big', bufs=1) as big,
            tc.tile_pool(name='scratch', bufs=2) as scratch,
            tc.tile_pool(name='scb', bufs=4) as scbp,
            tc.tile_pool(name='apool', bufs=4) as apool,
            tc.tile_pool(name='psSP', bufs=3, space='PSUM') as psSP,
            tc.tile_pool(name='psAcc', bufs=2, space='PSUM') as psAcc,
        ):
            # ---- loads ----
            x0 = big.tile([P, L], f32, tag='x0')
            x1 = big.tile([P, L], f32, tag='x1')
            nc.sync.dma_start(out=x0, in_=xd[0:P, :])
            nc.sync.dma_start(out=x1, in_=xd[P:C, :])

            wq_nat = const.tile([HID, C], f32, tag='wq_nat')
            wk_nat = const.tile([HID, C], f32, tag='wk_nat')
            wv_nat = const.tile([HID, C], f32, tag='wv_nat')
            wo_nat = [const.tile([P, HID], f32, tag=f'wo_nat{c}', name=f'wo_nat{c}') for c in range(2)]
            nc.sync.dma_start(out=wq_nat, in_=wqd)
            nc.sync.dma_start(out=wk_nat, in_=wkd)
            nc.sync.dma_start(out=wv_nat, in_=wvd)
            for c in range(2):
                nc.sync.dma_start(out=wo_nat[c], in_=wod[c * P:(c + 1) * P, :])

            gc = [const.tile([P, 1], f32, tag=f'g{c}', name=f'g{c}') for c in range(2)]
            bc = [const.tile([P, 1], f32, tag=f'b{c}', name=f'b{c}') for c in range(2)]
            boc = [const.tile([P, 1], f32, tag=f'bo{c}', name=f'bo{c}') for c in range(2)]
            for c in range(2):
                nc.sync.dma_start(out=gc[c], in_=gd[c * P:(c + 1) * P, :])
                nc.sync.dma_start(out=bc[c], in_=bd[c * P:(c + 1) * P, :])
                nc.sync.dma_start(out=boc[c], in_=bod[c * P:(c + 1) * P, :])

            ident = const.tile([P, P], f32, tag='ident')
            make_identity(nc, ident)
            ones_b = const.tile([P, P], bf16, tag='ones_b')
            nc.gpsimd.memset(ones_b, 1.0)
            epst = const.tile([P, 1], f32, tag='epst')
            nc.vector.memset(epst, EPS)
            neg_gc = [const.tile([P, 1], f32, tag=f'ng{c}', name=f'ng{c}') for c in range(2)]
            for c in range(2):
                nc.vector.tensor_scalar_mul(neg_gc[c], gc[c], -1.0)

            # ---- weight prep ----
            # QKV weights: transpose to [C, HID], scale by g, cast to bf16
            wT = {}
            for name, nat in (('q', wq_nat), ('k', wk_nat), ('v', wv_nat)):
                for c in range(2):
                    tp = psAcc.tile([P, P], f32, tag='fold')
                    nc.tensor.transpose(tp, nat[:, c * P:(c + 1) * P], ident)
                    t = const.tile([P, HID], bf16, tag=f'w{name}T{c}', name=f'w{name}T{c}')
                    nc.vector.tensor_scalar_mul(t, tp, gc[c])
                    wT[(name, c)] = t

            # Wo^T in pair layout: woT_pair[i] [128, 256] bf16 with rows 0-31 =
            # head 2i channels, rows 64-95 = head 2i+1 channels, rest zero
            # (zeros null out the Z/garbage bands of onorm in the out-proj matmul).
            woT_pair = [const.tile([P, C], bf16, tag=f'woT_pair{i}', name=f'woT_pair{i}') for i in range(2)]
            wtmp = [const.tile([32, C], bf16, tag=f'wtmp{i}', name=f'wtmp{i}') for i in range(2)]
            for i in range(2):
                nc.vector.memset(woT_pair[i], 0.0)
            for c in range(2):
                for i in range(2):
                    tp = psAcc.tile([32, P], f32, tag='fold')
                    nc.tensor.transpose(tp, wo_nat[c][:, (2 * i) * 32:(2 * i + 1) * 32], ident)
                    nc.vector.tensor_copy(woT_pair[i][0:32, c * P:(c + 1) * P], tp)
                    tp2 = psAcc.tile([32, P], f32, tag='fold')
                    nc.tensor.transpose(tp2, wo_nat[c][:, (2 * i + 1) * 32:(2 * i + 2) * 32], ident)
                    nc.vector.tensor_copy(wtmp[i][:, c * P:(c + 1) * P], tp2)
            for i in range(2):
                nc.sync.dma_start(out=woT_pair[i][64:96, :], in_=wtmp[i])

            # ---- augmentation rows: row0 = -s?g, row1 = b?  ----
            augin = []
            for c in range(2):
                ai = const.tile([P, 2], bf16, tag=f'augin{c}', name=f'augin{c}')
                nc.vector.tensor_copy(ai[:, 0:1], neg_gc[c])
                nc.vector.tensor_copy(ai[:, 1:2], bc[c])
                augin.append(ai)
            augT = {}
            for name in ('q', 'k', 'v'):
                ap_ = psAcc.tile([2, P], f32, tag='fold', name=f'augps{name}')
                for c in range(2):
                    nc.tensor.matmul(ap_, lhsT=augin[c], rhs=wT[(name, c)],
                                     start=(c == 0), stop=(c == 1))
                t = const.tile([2, P], bf16, tag=f'augT{name}', name=f'augT{name}')
                nc.vector.tensor_copy(t, ap_)
                augT[name] = t

            # ---- LayerNorm statistics (replicated across partitions via ones-matmul) ----
            xb0 = scbp.tile([P, L], bf16, tag='scb')
            xb1 = scbp.tile([P, L], bf16, tag='scb')
            nc.vector.tensor_copy(xb0, x0)
            nc.vector.tensor_copy(xb1, x1)
            xsq0 = scbp.tile([P, L], bf16, tag='scb')
            xsq1 = scbp.tile([P, L], bf16, tag='scb')
            nc.vector.tensor_mul(xsq0, xb0, xb0)
            nc.vector.tensor_mul(xsq1, xb1, xb1)

            mean_bc = big.tile([P, L], f32, tag='mean')
            veps = scratch.tile([P, L], f32, tag='sc')
            msq = scratch.tile([P, L], f32, tag='sc')
            for n in range(2):
                hs = slice(n * 1024, (n + 1) * 1024)
                s1p = psSP.tile([P, 1024], f32, tag='sp')
                for m in range(2):
                    sl = slice(m * 512, (m + 1) * 512)
                    gsl = slice(n * 1024 + m * 512, n * 1024 + (m + 1) * 512)
                    nc.tensor.matmul(s1p[:, sl], lhsT=ones_b, rhs=xb0[:, gsl], start=True, stop=False)
                    nc.tensor.matmul(s1p[:, sl], lhsT=ones_b, rhs=xb1[:, gsl], start=False, stop=True)
                nc.vector.tensor_scalar_mul(mean_bc[:, hs], s1p, 1.0 / C)
                nc.vector.tensor_mul(msq[:, hs], mean_bc[:, hs], mean_bc[:, hs])
                s2p = psSP.tile([P, 1024], f32, tag='sp')
                for m in range(2):
                    sl = slice(m * 512, (m + 1) * 512)
                    gsl = slice(n * 1024 + m * 512, n * 1024 + (m + 1) * 512)
                    nc.tensor.matmul(s2p[:, sl], lhsT=ones_b, rhs=xsq0[:, gsl], start=True, stop=False)
                    nc.tensor.matmul(s2p[:, sl], lhsT=ones_b, rhs=xsq1[:, gsl], start=False, stop=True)
                nc.vector.scalar_tensor_tensor(veps[:, hs], in0=s2p, scalar=1.0 / C,
                                               in1=msq[:, hs], op0=OP.mult, op1=OP.subtract)
            # rstd = exp(-0.5*ln(var+eps)) — stays in the ln/exp table set
            lnv = scratch.tile([P, L], f32, tag='sc')
            nc.scalar.activation(lnv, veps, AF.Ln, bias=epst)
            rstd_bc = big.tile([P, L], f32, tag='rstd')
            nc.scalar.activation(rstd_bc, lnv, AF.Exp, scale=-0.5)

            xs0 = big.tile([P, L], bf16, tag='xs0')
            xs1 = big.tile([P, L], bf16, tag='xs1')
            nc.vector.tensor_mul(xs0, x0, rstd_bc)
            nc.vector.tensor_mul(xs1, x1, rstd_bc)
            aug2 = const.tile([2, L], bf16, tag='aug2')
            nc.gpsimd.memset(aug2, 1.0)
            nc.vector.tensor_mul(aug2[0:1, :], mean_bc[0:1, :], rstd_bc[0:1, :])

            # ---- QKV projections (feature-major QT/KT; V position-major with ones fold) ----
            qt = big.tile([HID, L], bf16, tag='qt')
            kt = big.tile([HID, L], bf16, tag='kt')
            for name, dst in (('q', qt), ('k', kt)):
                for n in range(2):
                    pp = psSP.tile([P, 1024], f32, tag='sp')
                    for m in range(2):
                        sl = slice(m * 512, (m + 1) * 512)
                        gsl = slice(n * 1024 + m * 512, n * 1024 + (m + 1) * 512)
                        nc.tensor.matmul(pp[:, sl], lhsT=wT[(name, 0)], rhs=xs0[:, gsl], start=True, stop=False)
                        nc.tensor.matmul(pp[:, sl], lhsT=wT[(name, 1)], rhs=xs1[:, gsl], start=False, stop=False)
                        nc.tensor.matmul(pp[:, sl], lhsT=augT[name], rhs=aug2[:, gsl], start=False, stop=True)
                    nc.vector.tensor_copy(dst[:, n * 1024:(n + 1) * 1024], pp)

            # vext[:, e, h, 0:32] = V channels of head h at positions of block e;
            # vext[:, e, h, 32:64] = 1.0 (the Z fold columns)
            vext = big.tile([P, NE, H, 64], bf16, tag='vext')
            nc.gpsimd.memset(vext, 1.0)
            for e in range(NE):
                se = slice(e * P, (e + 1) * P)
                vp = psSP.tile([P, HID], f32, tag='sp')
                nc.tensor.matmul(vp, lhsT=xs0[:, se], rhs=wT[('v', 0)], start=True, stop=False)
                nc.tensor.matmul(vp, lhsT=xs1[:, se], rhs=wT[('v', 1)], start=False, stop=False)
                nc.tensor.matmul(vp, lhsT=aug2[:, se], rhs=augT['v'], start=False, stop=True)
                nc.vector.tensor_copy(vext[:, e, :, 0:32], vp)

            # static tiles for the per-d tail (pair layout: rows 0-31 op_even,
            # 32-63 Z_even, 64-95 op_odd, 96-127 Z_odd)
            accsb = [const.tile([P, 512], f32, tag=f'accsb{i}', name=f'accsb{i}') for i in range(2)]
            rcp = [const.tile([P, 512], f32, tag=f'rcp{i}', name=f'rcp{i}') for i in range(2)]
            rz = [const.tile([P, 512], f32, tag=f'rz{i}', name=f'rz{i}') for i in range(2)]
            onorm = [const.tile([P, 512], bf16, tag=f'onorm{i}', name=f'onorm{i}') for i in range(2)]
            for i in range(2):
                nc.vector.memset(rz[i], 1.0)

            # PE warmup burst: ~4us of dense full-array matmuls right before
            # the attention loop so the HAM un-throttles the PE clock to 8/8
            # (2.4 GHz) — the attention phase alone never has a 3.4us
            # continuous-busy window to trigger it, but also never idles long
            # enough to re-throttle, so the state at entry is sticky.
            # Depends on qt/kt so the scheduler cannot hoist it into the
            # earlier DVE-heavy phases; the result is never read.
            warm = psSP.tile([P, 512], f32, tag='sp')
            for w in range(8):
                nc.tensor.matmul(warm, lhsT=kt[:, 0:128], rhs=qt[:, 0:512],
                                 start=(w == 0), stop=(w == 7))

            # ---- attention ----
            for d in range(NL):
                sd = slice(d * 512, (d + 1) * 512)
                # pair fold accumulators [128, 512]: head 2i at rows 0-63
                # (op 0-31, Z 32-63), head 2i+1 at rows 64-127 via tile_position
                acc = [psAcc.tile([P, 512], f32, tag='fold', name=f'acc{d}_{i}') for i in range(2)]
                for e in range(NE):
                    se = slice(e * P, (e + 1) * P)
                    for pair in range(2):
                        h0, h1 = 2 * pair, 2 * pair + 1
                        sp = psSP.tile([P, 1024], f32, tag='sp')
                        nc.tensor.matmul(sp[:, 0:512], lhsT=kt[h0 * 32:(h0 + 1) * 32, se],
                                         rhs=qt[h0 * 32:(h0 + 1) * 32, sd],
                                         start=True, stop=True, tile_position=(32 * h0, 0))
                        nc.tensor.matmul(sp[:, 512:1024], lhsT=kt[h1 * 32:(h1 + 1) * 32, se],
                                         rhs=qt[h1 * 32:(h1 + 1) * 32, sd],
                                         start=True, stop=True, tile_position=(32 * h1, 0))
                        at = apool.tile([P, 1024], bf16, tag='at')
                        nc.scalar.activation(at, sp, AF.Exp, scale=SCALE)
                        nc.tensor.matmul(acc[pair][0:64, :], lhsT=vext[:, e, h0, :],
                                         rhs=at[:, 0:512],
                                         start=(e == 0), stop=(e == NE - 1),
                                         tile_position=(0, 0))
                        nc.tensor.matmul(acc[pair][64:128, :], lhsT=vext[:, e, h1, :],
                                         rhs=at[:, 512:1024],
                                         start=(e == 0), stop=(e == NE - 1),
                                         tile_position=(0, 64))
                # tail: copy psum out early (frees acc for d+1), 1/Z via fast
                # approx, DMA-realign Z bands down 32 partitions, normalize
                for i in range(2):
                    nc.vector.tensor_copy(accsb[i], acc[i])
                    nc.vector.reciprocal_approx_fast(out=rcp[i], in_=accsb[i])
                    nc.sync.dma_start(out=rz[i][0:32, :], in_=rcp[i][32:64, :])
                    nc.sync.dma_start(out=rz[i][64:96, :], in_=rcp[i][96:128, :])
                    nc.vector.tensor_mul(onorm[i], accsb[i], rz[i])
                for c in range(2):
                    yp = psSP.tile([P, 512], f32, tag='sp')
                    nc.tensor.matmul(yp, lhsT=woT_pair[0][:, c * P:(c + 1) * P],
                                     rhs=onorm[0], start=True, stop=False)
                    nc.tensor.matmul(yp, lhsT=woT_pair[1][:, c * P:(c + 1) * P],
                                     rhs=onorm[1], start=False, stop=True)
                    ysb = scratch.tile([P, 512], f32, tag='ysb')
                    xc = x0 if c == 0 else x1
                    nc.vector.scalar_tensor_tensor(ysb, in0=yp, scalar=boc[c],
                                                   in1=xc[:, sd], op0=OP.add, op1=OP.add)
                    nc.sync.dma_start(out=yd[c * P:(c + 1) * P, sd], in_=ysb)

    nc.compile()
    return nc


def _get_nc():
    global _cached
    if _cached is None:
        _cached = _build()
    return _cached


def kernel(**inputs):
    from concourse.bass_utils import run_bass_kernel_spmd

    x = np.ascontiguousarray(np.asarray(inputs['x'], dtype=np.float32))
    g = np.asarray(inputs['g'], dtype=np.float32).reshape(C, 1)
    b = np.asarray(inputs['b'], dtype=np.float32).reshape(C, 1)
    wq = np.ascontiguousarray(np.asarray(inputs['Wq'], dtype=np.float32))
    wk = np.ascontiguousarray(np.asarray(inputs['Wk'], dtype=np.float32))
    wv = np.ascontiguousarray(np.asarray(inputs['Wv'], dtype=np.float32))
    wo = np.ascontiguousarray(np.asarray(inputs['Wo'], dtype=np.float32))
    bo = np.asarray(inputs['bo'], dtype=np.float32).reshape(C, 1)

    nc = _get_nc()
    in_maps = [
        {'x': x[i], 'g': g, 'b': b, 'Wq': wq, 'Wk': wk, 'Wv': wv, 'Wo': wo, 'bo': bo}
        for i in range(B)
    ]
    res = run_bass_kernel_spmd(nc, in_maps, list(range(B)))
    return np.stack([res.results[i]['y'] for i in range(B)]).astype(np.float32)
